# revision 14
# baseline (speedup 1.0000x reference)
"""CrossAttention Trainium2 kernel (v3: bf16 pipeline, l-major AV).

Reference (per batch b): q = x@Wq; k = ctx@Wk; v = ctx@Wv (H=8, DH=64)
  out = softmax(q k^T / sqrt(DH)) v @ Wo + bo, rows >= seq_len zeroed.

Valid 512-row query tiles are packed across 8 cores with a per-core slot
structure: each core runs CAP tiles; slot j holds sizes[j] consecutive
tiles reading KV buffer j (which batch a (core, slot) holds is data, so
one SPMD program serves all cores). A planner picks slot sizes (up to 3
slots) minimizing CAP then slots; for the staged seq_lens it packs 56
tiles as 8 cores x 7 tiles with zero slot waste.

Engine plan (per tile of 512 queries):
- PE (bf16, 1 cyc/row): qT 4096 + scores 16384 + AV 8320 + transpose
  2048 + oproj 4096 cycles; KV 24576/slot.
- ACT: exp psum->bf16, 16 instrs of [128,2,512] (~16.6us) - bottleneck;
  scores for tile t+1 are emitted before the AV tail of tile t so the
  ACT queue never starves.
- AV is l-major (out [l, h, 65], v augmented with a ones column) so the
  softmax denominator lands as a per-partition scalar: reciprocal
  [128,4,1] + broadcast_to multiply on DVE; then PE-transpose (identity
  matmul) to outT for the bf16 output projection (+ K=1 bias row).
fp8 was measured (numpy mirror of this exact dataflow) at 4e-2..8e-2
rel err - above the 2e-2 gate - so everything stays bf16 (4.9e-3).
"""

import math
import sys

sys.path.insert(0, "/opt/trn_rl_repo")

import numpy as np
import ml_dtypes

B, L, S = 8, 8192, 512
DQ, DC = 256, 768
H, DH = 8, 64
INNER = H * DH
TL = 512
N_CORES = 8

BF16 = ml_dtypes.bfloat16
EXP_SCALE = 1.0 / 8.0          # 1/sqrt(DH)


def _plan(nt):
    """Pick slot sizes (<=3 slots) and per-core pieces.

    Returns (sizes, cores): sizes = tuple of slot lengths (tiles); cores =
    list of 8 entries, each a list of len(sizes) pieces (batch, tile0),
    batch -1 = padding."""
    T = sum(nt)
    lo = max(1, math.ceil(T / N_CORES))
    order = sorted(range(len(nt)), key=lambda i: -nt[i])

    def combos(n, sizes):
        k = len(sizes)
        out = []
        maxx = [min(8, math.ceil(n / s) + 1) for s in sizes]

        def rec(j, x, tot):
            if j == k:
                if tot >= n and all(
                    x[i] == 0 or tot - sizes[i] < n for i in range(k)
                ):
                    out.append(tuple(x))
                return
            for c in range(0, maxx[j] + 1):
                x.append(c)
                rec(j + 1, x, tot + c * sizes[j])
                x.pop()

        rec(0, [], 0)
        return out

    def feasible(sizes):
        k = len(sizes)
        budget = [N_CORES] * k
        pick = [None] * len(nt)

        def rec(bi):
            if bi == len(nt):
                return True
            i = order[bi]
            for x in combos(nt[i], sizes):
                if all(budget[j] >= x[j] for j in range(k)):
                    for j in range(k):
                        budget[j] -= x[j]
                    pick[i] = x
                    if rec(bi + 1):
                        return True
                    for j in range(k):
                        budget[j] += x[j]
            return False

        return pick if rec(0) else None

    best = None
    for cap in range(lo, lo + 10):
        for k in (1, 2, 3):
            parts = set()
            if k == 1:
                parts.add((cap,))
            elif k == 2:
                for a in range(cap - 1, 0, -1):
                    if a >= cap - a:
                        parts.add((a, cap - a))
            else:
                for a in range(cap - 2, 0, -1):
                    for b in range(min(a, cap - a - 1), 0, -1):
                        c = cap - a - b
                        if 0 < c <= b:
                            parts.add((a, b, c))
            for sizes in sorted(parts, reverse=True):
                pick = feasible(sizes)
                if pick is not None:
                    best = (sizes, pick)
                    break
            if best:
                break
        if best:
            break
    assert best is not None
    sizes, pick = best
    k = len(sizes)
    slot_pieces = [[] for _ in range(k)]
    for i, n in enumerate(nt):
        off = 0
        for j in range(k):
            for _ in range(pick[i][j]):
                slot_pieces[j].append((i, off))
                off += sizes[j]
    for j in range(k):
        while len(slot_pieces[j]) < N_CORES:
            slot_pieces[j].append((-1, 0))
    cores = [[slot_pieces[j][c] for j in range(k)] for c in range(N_CORES)]
    return sizes, cores


_PROG_CACHE = {}


def _build_program(sizes, has_bias):
    import concourse.mybir as mybir
    import concourse.tile as tile
    from concourse import bacc

    f32 = mybir.dt.float32
    bf16 = mybir.dt.bfloat16
    Exp = mybir.ActivationFunctionType.Exp
    NSLOT = len(sizes)
    CAP = sum(sizes)
    slot_of = []
    for j, s in enumerate(sizes):
        slot_of += [j] * s

    nc = bacc.Bacc("TRN2", target_bir_lowering=False, debug=False,
                   num_devices=N_CORES)
    x16 = nc.declare_dram_parameter("x16", [128, 2, CAP, TL], bf16,
                                    isOutput=False)
    ctxs = [nc.declare_dram_parameter(f"ctx{j}", [128, 6, S], bf16,
                                      isOutput=False) for j in range(NSLOT)]
    wq16 = nc.declare_dram_parameter("wq16", [128, 2, INNER], bf16,
                                     isOutput=False)
    wk16 = nc.declare_dram_parameter("wk16", [128, 6, INNER], bf16,
                                     isOutput=False)
    wv16 = nc.declare_dram_parameter("wv16", [128, 6, INNER], bf16,
                                     isOutput=False)
    wo16 = nc.declare_dram_parameter("wo16", [128, 4, DQ], bf16,
                                     isOutput=False)
    if has_bias:
        bo16 = nc.declare_dram_parameter("bo16", [1, DQ], bf16,
                                         isOutput=False)
    id16 = nc.declare_dram_parameter("id16", [128, 128], bf16,
                                     isOutput=False)
    y = nc.declare_dram_parameter("y", [CAP * TL, DQ], f32, isOutput=True)

    with tile.TileContext(nc) as tc:
        with (
            tc.tile_pool(name="wpool", bufs=1) as wpool,
            tc.tile_pool(name="kvpool", bufs=1) as kvpool,
            tc.tile_pool(name="mpool", bufs=3) as mpool,
            tc.tile_pool(name="qpool", bufs=2) as qpool,
            tc.tile_pool(name="epool", bufs=2) as epool,
            tc.tile_pool(name="apool", bufs=2) as apool,
            tc.tile_pool(name="opool", bufs=2) as opool,
            tc.tile_pool(name="spool", bufs=2) as spool,
            tc.tile_pool(name="ypool", bufs=3) as ypool,
            tc.tile_pool(name="ps_sc", bufs=2, space="PSUM") as ps_sc,
            tc.tile_pool(name="ps_av", bufs=1, space="PSUM") as ps_av,
            tc.tile_pool(name="ps_q", bufs=1, space="PSUM") as ps_q,
            tc.tile_pool(name="ps_tr", bufs=1, space="PSUM") as ps_tr,
        ):
            # ---- weights / constants (DMA order ~ first-use order) ----
            wq_sb = wpool.tile([128, 2, INNER], bf16, tag="wq", name="wq")
            nc.sync.dma_start(wq_sb[:], wq16[:])
            pre_x = {}
            for t in range(min(2, CAP)):
                xt = mpool.tile([128, 2, TL], bf16, tag="x", name="x")
                nc.sync.dma_start(xt[:], x16[:, :, t, :])
                pre_x[t] = xt
            wk_sb = wpool.tile([128, 6, INNER], bf16, tag="wk", name="wk")
            nc.sync.dma_start(wk_sb[:], wk16[:])
            ctx_sb = [kvpool.tile([128, 6, S], bf16, tag=f"ctx{j}",
                                  name=f"ctx{j}") for j in range(NSLOT)]
            nc.sync.dma_start(ctx_sb[0][:], ctxs[0][:])
            wv_sb = wpool.tile([128, 6, INNER], bf16, tag="wv", name="wv")
            nc.sync.dma_start(wv_sb[:], wv16[:])
            for j in range(1, NSLOT):
                nc.sync.dma_start(ctx_sb[j][:], ctxs[j][:])
            wo_sb = wpool.tile([128, 4, DQ], bf16, tag="wo", name="wo")
            nc.sync.dma_start(wo_sb[:], wo16[:])
            if has_bias:
                bo_sb = wpool.tile([1, DQ], bf16, tag="bo", name="bo")
                nc.sync.dma_start(bo_sb[:], bo16[:])
                ones1 = wpool.tile([1, 128], bf16, tag="on", name="on")
                nc.gpsimd.memset(ones1[:], 1.0)
            id_sb = wpool.tile([128, 128], bf16, tag="id", name="id")
            nc.sync.dma_start(id_sb[:], id16[:])

            # ---- KV pieces per slot (emitted chunked via the work queue) ----
            kT = {}
            vT = {}

            def kv_alloc(j):
                # kT[j]: [128(par*64+d), 4(hpair), S] bf16
                kT[j] = kvpool.tile([128, 4, S], bf16, tag=f"kT{j}",
                                    name=f"kT{j}")
                # vT[j]: [128(s), 4(schunk), 8(h), 65] bf16, col 64 = 1.0
                vT[j] = kvpool.tile([128, 4, 8, DH + 1], bf16, tag=f"v{j}",
                                    name=f"v{j}")
                nc.gpsimd.memset(vT[j][:, :, :, DH:DH + 1], 1.0)

            def kv_kt_chunk(j, m):
                pk = ps_q.tile([128, TL], f32, tag="pq", name="pq")
                for kc in range(6):
                    nc.tensor.matmul(
                        pk[:], wk_sb[:, kc, m * 128:(m + 1) * 128],
                        ctx_sb[j][:, kc, :], start=(kc == 0), stop=(kc == 5))
                nc.vector.tensor_copy(kT[j][:, m, :], pk[:])

            def kv_v_chunk(j, sc):
                pv = ps_q.tile([128, TL], f32, tag="pq", name="pq")
                for kc in range(6):
                    nc.tensor.matmul(
                        pv[:], ctx_sb[j][:, kc, sc * 128:(sc + 1) * 128],
                        wv_sb[:, kc, :], start=(kc == 0), stop=(kc == 5))
                nc.vector.tensor_copy(
                    vT[j][:, sc, :, 0:DH],
                    pv[:].rearrange("p (h d) -> p h d", d=DH))

            # ---- per-tile pieces ----
            def qt_alloc():
                return [qpool.tile([128, TL], bf16, tag=f"q{m}",
                                   name=f"q{m}") for m in range(4)]

            def qt_chunk(qT, xt, m):
                pq = ps_q.tile([128, TL], f32, tag="pq", name="pq")
                for kc in range(2):
                    nc.tensor.matmul(
                        pq[:], wq_sb[:, kc, m * 128:(m + 1) * 128],
                        xt[:, kc, :], start=(kc == 0), stop=(kc == 1))
                nc.vector.tensor_copy(qT[m][:], pq[:])

            def scores_head(j, qT, e, h):
                c, par = h // 2, h % 2
                rhs = qT[c][64 * par:64 * par + 64, :]
                eh = []
                for g in range(2):
                    psc = ps_sc.tile([128, 2, TL], f32, tag="sc", name="sc")
                    for i in range(2):
                        sc = 2 * g + i
                        nc.tensor.matmul(
                            psc[:, i, :],
                            kT[j][64 * par:64 * par + 64, c,
                                  sc * 128:(sc + 1) * 128],
                            rhs, start=True, stop=True)
                    e16 = epool.tile([128, 2, TL], bf16, tag=f"e{h}{g}",
                                     name=f"e{h}{g}")
                    nc.scalar.activation(e16[:], psc[:], Exp,
                                         scale=EXP_SCALE)
                    eh.append(e16)
                e.append(eh)

            # tail chunks for tile t (13 closures)
            def tail_chunks(t, j, e):
                st = {}

                def av_c(lc):
                    pav = [ps_av.tile([128, 4, DH + 1], f32, tag=f"va{a}",
                                      name=f"va{a}") for a in range(2)]
                    st[lc] = pav
                    for h in range(H):
                        a, hh = h // 4, h % 4
                        for sc in range(4):
                            nc.tensor.matmul(
                                pav[a][:, hh, :],
                                e[h][sc // 2][:, sc % 2,
                                              lc * 128:(lc + 1) * 128],
                                vT[j][:, sc, h, :],
                                start=(sc == 0), stop=(sc == 3))

                def norm_c(lc):
                    pav = st.pop(lc)
                    rp = spool.tile([128, 8, 1], f32, tag="rp", name="rp")
                    av = apool.tile([128, 8, DH], bf16, tag=f"av{lc}",
                                    name=f"av{lc}")
                    st[("av", lc)] = av
                    for a in range(2):
                        nc.vector.reciprocal(rp[:, 4 * a:4 * a + 4, :],
                                             pav[a][:, :, DH:DH + 1])
                        nc.vector.tensor_tensor(
                            av[:, 4 * a:4 * a + 4, :], pav[a][:, :, 0:DH],
                            rp[:, 4 * a:4 * a + 4, :].broadcast_to(
                                [128, 4, DH]),
                            mybir.AluOpType.mult)

                def tr_c(lc):
                    av = st.pop(("av", lc))
                    ptr = ps_tr.tile([128, 4, 128], bf16, tag="tr",
                                     name="tr")
                    for ic in range(4):
                        nc.tensor.transpose(
                            ptr[:, ic, :], av[:, 2 * ic:2 * ic + 2, :],
                            id_sb[:])
                    nc.vector.tensor_copy(st["outT"][:, :, lc, :], ptr[:])

                def oproj_c(ls):
                    outT = st["outT"]
                    po = ps_q.tile([128, TL], f32, tag="pq", name="pq")
                    for kc in range(4):
                        nc.tensor.matmul(po[:, 0:DQ], outT[:, kc, ls, :],
                                         wo_sb[:, kc, :], start=(kc == 0),
                                         stop=(kc == 3 and not has_bias))
                    if has_bias:
                        nc.tensor.matmul(po[:, 0:DQ], ones1[:], bo_sb[:],
                                         start=False, stop=True)
                    yt = ypool.tile([128, DQ], f32, tag="y", name="y")
                    nc.vector.tensor_copy(yt[:], po[:, 0:DQ])
                    nc.sync.dma_start(
                        y[t * TL + ls * 128:t * TL + (ls + 1) * 128, :],
                        yt[:])

                def alloc_outT():
                    st["outT"] = opool.tile([128, 4, 4, 128], bf16,
                                            tag="oT", name="oT")

                return ([lambda lc=0: av_c(0)]
                        + [lambda lc=lc: (norm_c(lc - 1), av_c(lc))
                           for lc in range(1, 4)]
                        + [lambda: (norm_c(3), alloc_outT())]
                        + [lambda lc=lc: tr_c(lc) for lc in range(4)]
                        + [lambda ls=ls: oproj_c(ls) for ls in range(4)])

            # ---- main loop: deadline-ordered work weave ----
            import heapq
            work = []       # heap of (deadline, seq, closure)
            seqn = [0]

            def push(dl, fn):
                heapq.heappush(work, (dl, seqn[0], fn))
                seqn[0] += 1

            def pump(n):
                for _ in range(n):
                    if not work:
                        return
                    heapq.heappop(work)[2]()

            def drain(i):
                while work and work[0][0] <= i:
                    heapq.heappop(work)[2]()

            first_tile = {}
            for t in range(CAP):
                first_tile.setdefault(slot_of[t], t)

            # startup: kv0 kT interleaved with scores(0) heads; vT(0) queued
            kv_alloc(0)
            for sc in range(4):
                push(1, lambda j=0, sc=sc: kv_v_chunk(j, sc))
            # enqueue later slots' kv early (deadline = their first tile)
            for j in range(1, NSLOT):
                kv_alloc(j)
                ft = first_tile[j]
                for m in range(4):
                    push(ft, lambda j=j, m=m: kv_kt_chunk(j, m))
                for sc in range(4):
                    push(ft, lambda j=j, sc=sc: kv_v_chunk(j, sc))

            qT_cur = qt_alloc()
            for m in range(4):
                qt_chunk(qT_cur, pre_x[0], m)

            for t in range(CAP):
                j = slot_of[t]
                drain(t)
                if t + 2 < CAP:
                    xt2 = mpool.tile([128, 2, TL], bf16, tag="x", name="x")
                    nc.sync.dma_start(xt2[:], x16[:, :, t + 2, :])
                    pre_x[t + 2] = xt2
                if t + 1 < CAP:
                    qT_nxt = qt_alloc()
                    xt_n = pre_x[t + 1]
                    for m in range(4):
                        push(t + 1,
                             lambda q=qT_nxt, x=xt_n, m=m: qt_chunk(q, x, m))
                e = []
                for h in range(H):
                    if t == 0 and h % 2 == 0:
                        kv_kt_chunk(0, h // 2)
                    scores_head(j, qT_cur, e, h)
                    if h >= 1:
                        n = 2 if len(work) <= 2 * (H - h) else 3
                        pump(n)
                for c in tail_chunks(t, j, e):
                    push(t + 2, c)
                if t + 1 < CAP:
                    qT_cur = qT_nxt
            drain(CAP + 2)
    nc.compile()
    return nc


def kernel(x, context, seq_lens, Wq, Wk, Wv, Wo, bo):
    from concourse.bass_utils import run_bass_kernel_spmd

    x = np.asarray(x, dtype=np.float32)
    context = np.asarray(context, dtype=np.float32)
    seq_lens = np.asarray(seq_lens, dtype=np.int32)
    Wq = np.asarray(Wq, dtype=np.float32)
    Wk = np.asarray(Wk, dtype=np.float32)
    Wv = np.asarray(Wv, dtype=np.float32)
    Wo = np.asarray(Wo, dtype=np.float32)
    bo = np.asarray(bo, dtype=np.float32)

    lens = np.clip(seq_lens, 1, L)
    nt = [int(math.ceil(int(n) / TL)) for n in lens]
    sizes, cores = _plan(nt)
    NSLOT = len(sizes)
    CAP = sum(sizes)
    has_bias = bool(np.any(bo != 0.0))

    key = (sizes, has_bias)
    if key not in _PROG_CACHE:
        _PROG_CACHE[key] = _build_program(sizes, has_bias)
    nc = _PROG_CACHE[key]

    wq_in = np.ascontiguousarray(
        Wq.reshape(2, 128, INNER).transpose(1, 0, 2)).astype(BF16)
    wk_in = np.ascontiguousarray(
        Wk.reshape(6, 128, INNER).transpose(1, 0, 2)).astype(BF16)
    wv_in = np.ascontiguousarray(
        Wv.reshape(6, 128, INNER).transpose(1, 0, 2)).astype(BF16)
    wo_in = np.ascontiguousarray(
        Wo.reshape(4, 128, DQ).transpose(1, 0, 2)).astype(BF16)
    bo_in = bo[None, :].astype(BF16)
    id_in = np.eye(128, dtype=np.float32).astype(BF16)
    ctxT16 = {}
    for bi in range(B):
        ctxT16[bi] = np.ascontiguousarray(
            context[bi].T.reshape(6, 128, S).transpose(1, 0, 2)
        ).astype(BF16)

    in_maps = []
    for core in range(N_CORES):
        xt_core = np.zeros((CAP * TL, DQ), dtype=np.float32)
        m = {}
        off = 0
        for j in range(NSLOT):
            bi, t0 = cores[core][j]
            if bi >= 0:
                r0 = t0 * TL
                r1 = min(r0 + sizes[j] * TL, L)
                if r1 > r0:
                    xt_core[off:off + (r1 - r0)] = x[bi, r0:r1]
                m[f"ctx{j}"] = ctxT16[bi]
            else:
                m[f"ctx{j}"] = ctxT16[0]
            off += sizes[j] * TL
        m["x16"] = np.ascontiguousarray(
            xt_core.reshape(CAP, TL, 2, 128).transpose(3, 2, 0, 1)
        ).astype(BF16)
        m["wq16"] = wq_in
        m["wk16"] = wk_in
        m["wv16"] = wv_in
        m["wo16"] = wo_in
        if has_bias:
            m["bo16"] = bo_in
        m["id16"] = id_in
        in_maps.append(m)

    res = run_bass_kernel_spmd(nc, in_maps, list(range(N_CORES)))

    out = np.zeros((B, L, DQ), dtype=np.float32)
    for core in range(N_CORES):
        yc = res.results[core]["y"]
        off = 0
        for j in range(NSLOT):
            bi, t0 = cores[core][j]
            if bi >= 0:
                r0 = t0 * TL
                r1 = min(r0 + sizes[j] * TL, int(lens[bi]))
                if r1 > r0:
                    out[bi, r0:r1] = yc[off:off + (r1 - r0)]
            off += sizes[j] * TL
    return out
